# revision 15
# baseline (speedup 1.0000x reference)
"""Trainium2 Bass kernel: multi-head self-attention with RoPE + sigmoid gating.

Computes, for fixed shapes B=2, S=2048, E=1024, H=16, D=64:
    qkv = x @ w_qkv ; q,k roped (concatenated-halves layout)
    att = softmax(q k^T / sqrt(D)) ; out = (att @ v * sigmoid(x @ w_gate)) @ w_out + b_out

Sharding: 8 cores = 2 (batch) x 4 (head groups of 4 heads).  Each core computes a
row-parallel partial of the output projection for its batch (its 4 heads' slice of
the E contraction); the host sums the 4 partials per batch and adds b_out.

All matmuls run in bf16 (inputs pre-cast host-side); PSUM accumulation is fp32.
Softmax skips the max-subtraction (scores*scale are O(+-6), exp is safe in fp32)
so exp(scale*s) comes straight off the ScalarE activation LUT with the 1/8 scale
folded into the activation's free affine.  The softmax denominator rides along as
a 65th "ones" column of the PV matmul's stationary operand.  The denominator
reciprocal is partition-broadcast with a one-hot matmul, split hi/lo into two
bf16 matmuls (exactly the fp32 LOW_HIGH decomposition, 5x faster).

Schedule notes: per-engine instruction order is STATIC — emission order decides
it.  The kernel is jointly PE/ScalarE-bound, so emission interleaves the
projections into the attention stream, starts each chunk's score/exp pipeline
BEFORE the previous chunk's normalize/out-proj in the PE stream, and a 14-deep
ex pool rides the PSUM-bank hand-off gaps at chunk boundaries.

Device-side layouts (per core; host preps/permutes/casts all of these):
    xT   [1024, 2048]  x[b]^T
    wqk  [1024, 512]   columns: [q_even | q_odd | k_even | k_odd], each 128 = 4 heads x 32
    wv   [1024, 256]   v columns for the 4 heads (natural order)
    wg   [1024, 256]   w_gate columns for the 4 heads' output dims
    wo   [128, 2, 1024] w_out rows for the 4 heads, as 2 pair-tiles of 128
    cs/sn [128, 2048]  cos/sin RoPE tables, rows = 4x32 freqs, cols = position
Output: out [2048, 1024] fp32 partial (no bias).
"""

import numpy as np
import ml_dtypes

B, S, E, H, D = 2, 2048, 1024, 16, 64
HC = 4            # heads per core
NCORES = 8
KT = E // 128     # 8 contraction tiles
ST = S // 128     # 16 sequence tiles
SQ = 512          # attention sq chunk
NCH = S // SQ     # 4 chunks
ROPE_THETA = 10000.0

_CACHE = {}

# Results of the most recent kernel() call, for test harnesses.
LAST_RESULTS = None


# ---------------------------------------------------------------------------
# BIR postprocess: the walrus build in this image accepts only ONE sync-wait
# command per lowered TPB instruction (Drain/NoOp/LDWEIGHTS/...).  Tile emits
# instructions with several waits; split the excess onto preceding single-wait
# NoOps on the same engine (program order preserves the blocking semantics).
# Installed by patching concourse's compile_bir_kernel in this process.
# ---------------------------------------------------------------------------

def _split_waits(bir_bytes, limit=1):
    import json as _json
    m = _json.loads(bir_bytes)
    counter = [0]

    def fix_block(instrs):
        out = []
        for ins in instrs:
            w = ins.get("sync_info", {}).get("on_wait", [])
            if len(w) > limit:
                chunks = [w[i:i + limit] for i in range(0, len(w), limit)]
                ins["sync_info"]["on_wait"] = chunks[-1]
                for ch in chunks[:-1]:
                    counter[0] += 1
                    out.append({
                        "name": f"I-waitsplit-{counter[0]}",
                        "opcode": "NoOp",
                        "engine": ins.get("engine"),
                        "ins": [],
                        "outs": [],
                        "sync_info": {"on_update": [], "on_wait": ch},
                    })
            out.append(ins)
        return out

    def walk(d):
        if isinstance(d, dict):
            for k, v in d.items():
                if k == "instructions" and isinstance(v, list):
                    d[k] = fix_block(v)
                else:
                    walk(v)
        elif isinstance(d, list):
            for v in d:
                walk(v)

    walk(m)
    return _json.dumps(m).encode()


def _install_birfix():
    if _CACHE.get("birfix"):
        return
    _CACHE["birfix"] = True
    import concourse.bass_utils as bu
    import concourse.bass2jax as b2j

    orig = bu.compile_bir_kernel

    def patched(bir_json, tmpdir, neff_name="file.neff"):
        return orig(_split_waits(bir_json), tmpdir, neff_name=neff_name)

    bu.compile_bir_kernel = patched
    b2j.compile_bir_kernel = patched


def _build_nc():
    import concourse.bass as bass
    import concourse.mybir as mybir
    from concourse.tile import TileContext

    bf = mybir.dt.bfloat16
    f32 = mybir.dt.float32
    MUL = mybir.AluOpType.mult
    SUB = mybir.AluOpType.subtract
    ADD = mybir.AluOpType.add
    Act = mybir.ActivationFunctionType

    nc = bass.Bass()
    xT_d = nc.dram_tensor("xT", (E, S), bf, kind="ExternalInput")
    wqk_d = nc.dram_tensor("wqk", (E, 4 * 128), bf, kind="ExternalInput")
    wv_d = nc.dram_tensor("wv", (E, HC * 64), bf, kind="ExternalInput")
    wg_d = nc.dram_tensor("wg", (E, HC * 64), bf, kind="ExternalInput")
    wo_d = nc.dram_tensor("wo", (128, 2, E), bf, kind="ExternalInput")
    cs_d = nc.dram_tensor("cs", (128, S), bf, kind="ExternalInput")
    sn_d = nc.dram_tensor("sn", (128, S), bf, kind="ExternalInput")
    out_d = nc.dram_tensor("out", (S, E), f32, kind="ExternalOutput")

    scale = float(D) ** -0.5

    with TileContext(nc) as tc:
        with (
            tc.tile_pool(name="const", bufs=1) as cpool,
            tc.tile_pool(name="big", bufs=1) as bpool,
            tc.tile_pool(name="work", bufs=3) as wpool,
            tc.tile_pool(name="rope", bufs=2) as rpool,
            tc.tile_pool(name="expool", bufs=16) as expool,
            tc.tile_pool(name="outp", bufs=2) as opool,
            tc.tile_pool(name="scp", bufs=2, space="PSUM") as scpool,
            tc.tile_pool(name="pvp", bufs=1, space="PSUM") as pvpool,
        ):
            # ---- persistent tiles ----
            wqk = cpool.tile([128, KT, 512], bf)
            xT = bpool.tile([128, KT, S], bf)
            cs = cpool.tile([128, S], bf)
            sn = cpool.tile([128, S], bf)
            wv = cpool.tile([128, KT, 256], bf)
            wg = cpool.tile([128, KT, 256], bf)
            wo = cpool.tile([128, 2, E], bf)
            scr = cpool.tile([128, 512], bf)   # never written: PE warm-up fuel
            qR = [bpool.tile([128, S], bf, tag=f"qR{g}", name=f"qR{g}") for g in range(2)]
            kR = [bpool.tile([128, S], bf, tag=f"kR{g}", name=f"kR{g}") for g in range(2)]
            gP = [bpool.tile([128, S], bf, tag=f"gP{g}", name=f"gP{g}") for g in range(2)]
            ag = [bpool.tile([128, S], bf, tag=f"ag{g}", name=f"ag{g}") for g in range(2)]
            vOnes = [bpool.tile([128, HC * 65], bf, tag=f"vo{s}", name=f"vo{s}")
                     for s in range(ST)]

            # ---- PE warm-up: keep the HAM clock gate open until real MMs ----
            wps = scpool.tile([128, 512], f32, tag="sc", name="warm_mm")
            nc.gpsimd.memset(scr, 0.0)
            for i in range(32):
                nc.tensor.matmul(wps, lhsT=scr[:, 0:128], rhs=scr,
                                 start=(i == 0), stop=(i == 31))

            # ---- input DMAs, ordered so chunk-0 q proj can start ASAP ----
            nc.sync.dma_start(wqk[:, :, 0:256],
                              wqk_d[:, 0:256].rearrange("(k p) m -> p k m", p=128))
            nc.sync.dma_start(cs[:, 0:512], cs_d[:, 0:512])
            nc.sync.dma_start(sn[:, 0:512], sn_d[:, 0:512])
            nc.sync.dma_start(xT[:, 0:4, 0:512],
                              xT_d[0:512, 0:512].rearrange("(k p) s -> p k s", p=128))
            nc.sync.dma_start(xT[:, 4:8, 0:512],
                              xT_d[512:1024, 0:512].rearrange("(k p) s -> p k s", p=128))
            nc.sync.dma_start(wqk[:, :, 256:512],
                              wqk_d[:, 256:512].rearrange("(k p) m -> p k m", p=128))
            nc.sync.dma_start(wv, wv_d[:, :].rearrange("(k p) m -> p k m", p=128))
            nc.sync.dma_start(cs[:, 512:2048], cs_d[:, 512:2048])
            nc.sync.dma_start(sn[:, 512:2048], sn_d[:, 512:2048])
            nc.sync.dma_start(xT[:, :, 512:1024],
                              xT_d[:, 512:1024].rearrange("(k p) s -> p k s", p=128))
            nc.sync.dma_start(wg, wg_d[:, :].rearrange("(k p) m -> p k m", p=128))
            nc.sync.dma_start(xT[:, :, 1024:2048],
                              xT_d[:, 1024:2048].rearrange("(k p) s -> p k s", p=128))
            nc.sync.dma_start(wo, wo_d[:, :, :])

            # warm the ACT exp/tanh table set before it is on the critical path
            warm = cpool.tile([1, 8], f32)
            nc.vector.memset(warm, 0.0)
            nc.scalar.activation(warm, warm, Act.Exp)

            # one-hot rows for the matmul-based partition broadcast of the
            # softmax denominators: hot[32h, 128h + r] = 1
            hot = cpool.tile([128, HC * 128], bf)
            nc.vector.memset(hot, 0.0)
            for h in range(HC):
                nc.vector.memset(hot[32 * h:32 * h + 1, 128 * h:128 * (h + 1)], 1.0)

            # ---- emit helpers ----

            def emit_qk_slice(which, ssl):
                """Project+RoPE+assemble q or k for one 512-col seq slice.
                The two m-tiles' accumulation chains are interleaved so each
                matmul's LDWEIGHTS can prefetch into the background weight
                buffer while the other chain's matmul streams."""
                mbase = 0 if which == "q" else 2
                L = ssl.stop - ssl.start
                pss = [scpool.tile([128, L], f32, tag="sc", name=f"ps_{which}{mi}")
                       for mi in range(2)]
                for k in range(KT):
                    for mi in range(2):
                        nc.tensor.matmul(
                            pss[mi],
                            lhsT=wqk[:, k, (mbase + mi) * 128:(mbase + mi + 1) * 128],
                            rhs=xT[:, k, ssl],
                            start=(k == 0), stop=(k == KT - 1),
                        )
                raw = []
                for mi in range(2):
                    r = rpool.tile([128, L], bf, tag=f"raw{which}{mi}",
                                   name=f"raw{which}{mi}")
                    nc.vector.tensor_copy(r, pss[mi])
                    raw.append(r)
                ev, od = raw
                top = rpool.tile([128, L], bf, tag=f"top{which}", name=f"top{which}")
                bot = rpool.tile([128, L], bf, tag=f"bot{which}", name=f"bot{which}")
                t1 = wpool.tile([128, L], bf, tag="rt1", name="rt1", bufs=2)
                t2 = wpool.tile([128, L], bf, tag="rt2", name="rt2", bufs=2)
                nc.vector.tensor_tensor(t1, ev, cs[:, ssl], MUL)
                nc.vector.tensor_tensor(t2, od, sn[:, ssl], MUL)
                nc.vector.tensor_tensor(top, t1, t2, SUB)
                t3 = wpool.tile([128, L], bf, tag="rt1", name="rt3", bufs=2)
                t4 = wpool.tile([128, L], bf, tag="rt2", name="rt4", bufs=2)
                nc.vector.tensor_tensor(t3, ev, sn[:, ssl], MUL)
                nc.vector.tensor_tensor(t4, od, cs[:, ssl], MUL)
                nc.vector.tensor_tensor(bot, t3, t4, ADD)
                # assemble into qR/kR pair layout: rows 64*h2+{0:32 top, 32:64 bot}
                dsts = qR if which == "q" else kR
                for g in range(2):
                    for h2 in range(2):
                        h = 2 * g + h2
                        nc.sync.dma_start(dsts[g][64 * h2:64 * h2 + 32, ssl],
                                          top[32 * h:32 * h + 32, :])
                        nc.sync.dma_start(dsts[g][64 * h2 + 32:64 * h2 + 64, ssl],
                                          bot[32 * h:32 * h + 32, :])

            def emit_v(s0):
                """v projection for seq tiles (s0, s0+1) into [v_h | 1]
                stationary tiles; the two k-chains interleave for LDWEIGHTS
                background prefetch."""
                pair = (s0, s0 + 1)
                pss = []
                for s in pair:
                    nc.gpsimd.memset(vOnes[s], 1.0)
                    pss.append(scpool.tile([128, 512], f32, tag="sc", name="ps_v"))
                for k in range(KT):
                    for i, s in enumerate(pair):
                        nc.tensor.matmul(
                            pss[i][:, 0:256],
                            lhsT=xT[:, k, s * 128:(s + 1) * 128],
                            rhs=wv[:, k, :],
                            start=(k == 0), stop=(k == KT - 1),
                        )
                for i, s in enumerate(pair):
                    nc.vector.tensor_copy(
                        vOnes[s].rearrange("p (h w) -> p h w", w=65)[:, :, 0:64],
                        pss[i][:, 0:256].rearrange("p (h w) -> p h w", w=64),
                    )

            def emit_gate(g, n2):
                """gate projection (sigmoid via tanh) for one 1024-col slice.
                k-outer/half-inner so the stationary is reused by 2 matmuls."""
                sl = slice(n2 * 1024, (n2 + 1) * 1024)
                ps = scpool.tile([128, 1024], f32, tag="sc", name="ps_g")
                for k in range(KT):
                    for half in range(2):
                        o = 512 * half
                        nc.tensor.matmul(
                            ps[:, o:o + 512],
                            lhsT=wg[:, k, g * 128:(g + 1) * 128],
                            rhs=xT[:, k, n2 * 1024 + o:n2 * 1024 + o + 512],
                            start=(k == 0), stop=(k == KT - 1),
                        )
                th = wpool.tile([128, 1024], bf, tag="th", name="th", bufs=1)
                nc.scalar.activation(th, ps, Act.Tanh, scale=0.5)
                # sigmoid(x) = 0.5*tanh(x/2) + 0.5
                nc.vector.tensor_scalar(gP[g][:, sl], th, 0.5, 0.5, MUL, ADD)

            def emit_score_exp(c, sk):
                """scores+exp for one (chunk, key-tile); returns the ex pair."""
                csl = slice(c * SQ, (c + 1) * SQ)
                exs = []
                for g in range(2):
                    sct = scpool.tile([128, 1024], f32, tag="sc", name="sct")
                    for h2 in range(2):
                        nc.tensor.matmul(
                            sct[:, h2 * 512:(h2 + 1) * 512],
                            lhsT=kR[g][64 * h2:64 * (h2 + 1), sk * 128:(sk + 1) * 128],
                            rhs=qR[g][64 * h2:64 * (h2 + 1), csl],
                            start=True, stop=True,
                        )
                    ex = expool.tile([128, 1024], bf, tag="ex", name="ex")
                    nc.scalar.activation(ex, sct, Act.Exp, scale=scale)
                    exs.append(ex)
                return exs

            def emit_pv(sk, pv, exs):
                for g in range(2):
                    for h2 in range(2):
                        h = 2 * g + h2
                        nc.tensor.matmul(
                            pv[h][0:65, :],
                            lhsT=vOnes[sk][:, h * 65:(h + 1) * 65],
                            rhs=exs[g][:, h2 * 512:(h2 + 1) * 512],
                            start=(sk == 0), stop=(sk == ST - 1),
                        )

            def emit_att(c, sk, pv):
                emit_pv(sk, pv, emit_score_exp(c, sk))

            def emit_norm(c, pv, tail=False):
                """normalize + gate for one chunk -> ag pair tiles.
                Constraints honoured: engine ops need base partitions in
                {0,32,64,96}; both-SBUF inputs must share a base partition; at
                most one PSUM input.  The uu copies evacuate + release the pv
                banks first so the next chunk's PV accumulation restarts fast;
                recB reuses the pv-tag banks (the expool runway rides the gap).
                The reciprocal broadcast runs as two bf16 matmuls (hi + lo
                residual), the manual version of the fp32 LOW_HIGH split."""
                csl = slice(c * SQ, (c + 1) * SQ)
                dstack = wpool.tile([128, SQ], f32, tag="dstack", name="dstack", bufs=2)
                nc.vector.memset(dstack, 1.0)
                for h in range(HC):
                    # at the kernel tail ScalarE is idle: offload the small
                    # denominator-row moves there so DVE starts uu sooner
                    if tail and h % 2 == 1:
                        nc.scalar.copy(dstack[32 * h:32 * h + 1, :], pv[h][64:65, :])
                    else:
                        nc.vector.tensor_copy(dstack[32 * h:32 * h + 1, :],
                                              pv[h][64:65, :])
                uus = []
                for h in range(HC):
                    o = 64 * (h % 2)
                    uu = wpool.tile([128, SQ], f32, tag=f"uu{h}", name=f"uu{h}", bufs=1)
                    if tail and h >= 2:
                        nc.scalar.copy(uu[o:o + 64, :], pv[h][0:64, :])
                    else:
                        nc.vector.tensor_copy(uu[o:o + 64, :], pv[h][0:64, :])
                    uus.append(uu)
                rec128 = wpool.tile([128, SQ], f32, tag="rec128", name="rec128", bufs=2)
                nc.vector.reciprocal(out=rec128, in_=dstack)
                rhi = wpool.tile([128, SQ], bf, tag="rhi", name="rhi", bufs=2)
                rlo = wpool.tile([128, SQ], bf, tag="rlo", name="rlo", bufs=2)
                nc.vector.tensor_copy(rhi, rec128)
                nc.vector.tensor_tensor(rlo, rec128, rhi, SUB)
                for h in range(HC):
                    g, h2 = divmod(h, 2)
                    o = 64 * h2
                    recB = pvpool.tile([128, SQ], f32, tag=f"pv{h}", name="recB")
                    nc.tensor.matmul(recB, lhsT=hot[:, 128 * h:128 * (h + 1)],
                                     rhs=rhi, start=True, stop=False)
                    nc.tensor.matmul(recB, lhsT=hot[:, 128 * h:128 * (h + 1)],
                                     rhs=rlo, start=False, stop=True)
                    tmpu = wpool.tile([128, SQ], f32, tag="tmpu", name="tmpu", bufs=2)
                    nc.vector.tensor_tensor(tmpu[o:o + 64, :], uus[h][o:o + 64, :],
                                            recB[0:64, :], MUL)
                    nc.vector.tensor_tensor(
                        ag[g][o:o + 64, csl],
                        tmpu[o:o + 64, :],
                        gP[g][o:o + 64, csl],
                        MUL,
                    )

            def emit_out(c, tail=False):
                """output projection for one chunk's 4 row-tiles."""
                for st in range(SQ // 128):
                    s = (SQ // 128) * c + st
                    obs = []
                    for n in range(2):
                        ps = pvpool.tile([128, SQ], f32, tag=f"pv{n}", name="ps_o")
                        for g in range(2):
                            nc.tensor.matmul(
                                ps,
                                lhsT=ag[g][:, s * 128:(s + 1) * 128],
                                rhs=wo[:, g, n * 512:(n + 1) * 512],
                                start=(g == 0), stop=(g == 1),
                            )
                        obs.append(ps)
                    ob = opool.tile([128, 1024], f32, tag="ob", name="ob")
                    if tail:
                        nc.vector.tensor_copy(ob[:, 0:512], obs[0])
                        nc.scalar.copy(ob[:, 512:1024], obs[1])
                    else:
                        nc.vector.tensor_copy(ob[:, 0:512], obs[0])
                        nc.vector.tensor_copy(ob[:, 512:1024], obs[1])
                    nc.sync.dma_start(out_d[s * 128:(s + 1) * 128, :], ob)

            # ---- emission schedule ----
            # chunk-0 lead-in, software-pipelined one block ahead: block b's
            # score/exp tiles are emitted right after its k proj, while block
            # b-1's v proj + PV trail one slot behind.  exp for block b streams
            # while the PE grinds b-1's v/PV and b+1's k proj.  Gate tiles for
            # seq 0:1024 (needed by norm c0/c1) slot into the c0 stream; the
            # rest defers into the steady chunks.
            emit_qk_slice("q", slice(0, 512))
            pv_c = [pvpool.tile([128, SQ], f32, tag=f"pv{h}", name=f"pv{h}")
                    for h in range(HC)]
            prev_exs = None
            for b in range(4):
                emit_qk_slice("k", slice(b * 512, (b + 1) * 512))
                cur = [emit_score_exp(0, sk) for sk in range(4 * b, 4 * b + 4)]
                if prev_exs is not None:
                    pb = b - 1
                    emit_v(4 * pb)
                    emit_v(4 * pb + 2)
                    for i, sk in enumerate(range(4 * pb, 4 * pb + 4)):
                        emit_pv(sk, pv_c, prev_exs[i])
                if b == 1:
                    emit_gate(0, 0)
                if b == 2:
                    emit_qk_slice("q", slice(512, 1024))
                if b == 3:
                    emit_gate(1, 0)
                prev_exs = cur
            emit_v(12)
            emit_v(14)
            for i, sk in enumerate(range(12, 16)):
                emit_pv(sk, pv_c, prev_exs[i])

            # steady state: per chunk, the first 8 score/exp tiles are emitted
            # BEFORE the previous chunk's normalize+out-proj so the static PE
            # stream keeps feeding ScalarE across the boundary; the pv-tag
            # allocation order stays pv(c-1) -> recB(c-1) -> ps_o(c-1) -> pv(c).
            for c in range(1, NCH):
                exs = [emit_score_exp(c, sk) for sk in range(8)]
                emit_norm(c - 1, pv_c)
                emit_out(c - 1)
                pv_n = [pvpool.tile([128, SQ], f32, tag=f"pv{h}", name=f"pv{h}")
                        for h in range(HC)]
                for i in range(8):
                    emit_pv(i, pv_n, exs[i])
                for sk in range(8, ST):
                    emit_att(c, sk, pv_n)
                    if c == 1 and sk == 10:
                        emit_qk_slice("q", slice(1024, 1536))
                    if c == 2 and sk == 10:
                        emit_qk_slice("q", slice(1536, 2048))
                    if c == 1 and sk == 13:
                        emit_gate(0, 1)
                    if c == 2 and sk == 13:
                        emit_gate(1, 1)
                pv_c = pv_n
            emit_norm(NCH - 1, pv_c, tail=True)
            emit_out(NCH - 1, tail=True)

    return nc


def _host_inputs(x, w_qkv, w_gate, w_out):
    """Build the 8 per-core input maps (all device tensors bf16)."""
    bf = ml_dtypes.bfloat16
    x = np.asarray(x, dtype=np.float32)
    w_qkv = np.asarray(w_qkv, dtype=np.float32)
    w_gate = np.asarray(w_gate, dtype=np.float32)
    w_out = np.asarray(w_out, dtype=np.float32)

    inv = 1.0 / (ROPE_THETA ** (np.arange(0, D, 2, dtype=np.float64) / D))   # [32]
    ang = np.arange(S, dtype=np.float64)[None, :] * inv[:, None]             # [32, S]
    cs = np.tile(np.cos(ang), (4, 1)).astype(bf)                             # [128, S]
    sn = np.tile(np.sin(ang), (4, 1)).astype(bf)

    wq = w_qkv[:, 0:E]
    wk = w_qkv[:, E:2 * E]
    wvv = w_qkv[:, 2 * E:3 * E]

    in_maps = []
    for c in range(NCORES):
        b = c // 4
        hs = HC * (c % 4)
        cols_ev = np.concatenate([(hs + h) * 64 + np.arange(0, 64, 2) for h in range(HC)])
        cols_od = cols_ev + 1
        wqk_p = np.concatenate(
            [wq[:, cols_ev], wq[:, cols_od], wk[:, cols_ev], wk[:, cols_od]], axis=1)
        vcols = np.concatenate([(hs + h) * 64 + np.arange(64) for h in range(HC)])
        wo_p = w_out[vcols, :].reshape(2, 128, E).transpose(1, 0, 2)
        in_maps.append({
            "xT": np.ascontiguousarray(x[b].T).astype(bf),
            "wqk": np.ascontiguousarray(wqk_p).astype(bf),
            "wv": np.ascontiguousarray(wvv[:, vcols]).astype(bf),
            "wg": np.ascontiguousarray(w_gate[:, vcols]).astype(bf),
            "wo": np.ascontiguousarray(wo_p).astype(bf),
            "cs": cs,
            "sn": sn,
        })
    return in_maps


def kernel(x, w_qkv, w_gate, w_out, b_out, n_heads):
    global LAST_RESULTS
    assert int(n_heads) == H
    x = np.asarray(x)
    assert x.shape == (B, S, E)

    from concourse.bass_utils import run_bass_kernel_spmd

    _install_birfix()
    if "nc" not in _CACHE:
        _CACHE["nc"] = _build_nc()
    nc = _CACHE["nc"]

    in_maps = _host_inputs(x, w_qkv, w_gate, w_out)
    import os
    trace = bool(int(os.environ.get("KERNEL_TRACE", "0")))
    tmpdir = os.environ.get("KERNEL_TRACE_DIR") if trace else None
    res = run_bass_kernel_spmd(nc, in_maps, list(range(NCORES)), trace=trace,
                               tmpdir=tmpdir)
    LAST_RESULTS = res

    out = np.zeros((B, S, E), dtype=np.float32)
    for c in range(NCORES):
        out[c // 4] += res.results[c]["out"]
    out += np.asarray(b_out, dtype=np.float32)[None, None, :]
    return out


# revision 20
# speedup vs baseline: 1.1781x; 1.1781x over previous
"""Trainium2 Bass kernel: multi-head self-attention with RoPE + sigmoid gating.

Computes, for fixed shapes B=2, S=2048, E=1024, H=16, D=64:
    qkv = x @ w_qkv ; q,k roped (concatenated-halves layout)
    att = softmax(q k^T / sqrt(D)) ; out = (att @ v * sigmoid(x @ w_gate)) @ w_out + b_out

Sharding: 8 cores = 2 (batch) x 4 (head groups of 4 heads).  Each core computes a
row-parallel partial of the output projection for its batch (its 4 heads' slice of
the E contraction); the host sums the 4 partials per batch and adds b_out.

All matmuls run in bf16 (inputs pre-cast host-side); PSUM accumulation is fp32.
Softmax skips the max-subtraction (scores*scale are O(+-6), exp is safe in fp32)
so exp(scale*s) comes straight off the ScalarE activation LUT with the 1/8 scale
folded into the activation's free affine.  The softmax denominator rides along as
a 65th "ones" column of the PV matmul's stationary operand.  The denominator
reciprocal is partition-broadcast with a one-hot matmul, split hi/lo into two
bf16 matmuls (exactly the fp32 LOW_HIGH decomposition, 5x faster).

Schedule notes: per-engine instruction order is STATIC — emission order decides
it.  The kernel is jointly PE/ScalarE-bound, so emission interleaves the
projections into the attention stream, starts each chunk's score/exp pipeline
BEFORE the previous chunk's normalize/out-proj in the PE stream, and a 14-deep
ex pool rides the PSUM-bank hand-off gaps at chunk boundaries.

Device-side layouts (per core; host preps/permutes/casts all of these):
    xT   [1024, 2048]  x[b]^T
    wqk  [1024, 512]   columns: [q_even | q_odd | k_even | k_odd], each 128 = 4 heads x 32
    wv   [1024, 256]   v columns for the 4 heads (natural order)
    wg   [1024, 256]   w_gate columns for the 4 heads' output dims
    wo   [128, 2, 1024] w_out rows for the 4 heads, as 2 pair-tiles of 128
    cs/sn [128, 2048]  cos/sin RoPE tables, rows = 4x32 freqs, cols = position
Output: out [2048, 1024] fp32 partial (no bias).
"""

import numpy as np
import ml_dtypes

B, S, E, H, D = 2, 2048, 1024, 16, 64
HC = 4            # heads per core
NCORES = 8
KT = E // 128     # 8 contraction tiles
ST = S // 128     # 16 sequence tiles
SQ = 512          # attention sq chunk
NCH = S // SQ     # 4 chunks
ROPE_THETA = 10000.0

_CACHE = {}

# Results of the most recent kernel() call, for test harnesses.
LAST_RESULTS = None


# ---------------------------------------------------------------------------
# BIR postprocess: the walrus build in this image accepts only ONE sync-wait
# command per lowered TPB instruction (Drain/NoOp/LDWEIGHTS/...).  Tile emits
# instructions with several waits; split the excess onto preceding single-wait
# NoOps on the same engine (program order preserves the blocking semantics).
# Installed by patching concourse's compile_bir_kernel in this process.
# ---------------------------------------------------------------------------

def _split_waits(bir_bytes, limit=1):
    import json as _json
    m = _json.loads(bir_bytes)
    counter = [0]

    def fix_block(instrs):
        out = []
        for ins in instrs:
            w = ins.get("sync_info", {}).get("on_wait", [])
            if len(w) > limit:
                chunks = [w[i:i + limit] for i in range(0, len(w), limit)]
                ins["sync_info"]["on_wait"] = chunks[-1]
                for ch in chunks[:-1]:
                    counter[0] += 1
                    out.append({
                        "name": f"I-waitsplit-{counter[0]}",
                        "opcode": "NoOp",
                        "engine": ins.get("engine"),
                        "ins": [],
                        "outs": [],
                        "sync_info": {"on_update": [], "on_wait": ch},
                    })
            out.append(ins)
        return out

    def walk(d):
        if isinstance(d, dict):
            for k, v in d.items():
                if k == "instructions" and isinstance(v, list):
                    d[k] = fix_block(v)
                else:
                    walk(v)
        elif isinstance(d, list):
            for v in d:
                walk(v)

    walk(m)
    return _json.dumps(m).encode()


def _install_birfix():
    if _CACHE.get("birfix"):
        return
    _CACHE["birfix"] = True
    import concourse.bass_utils as bu
    import concourse.bass2jax as b2j

    orig = bu.compile_bir_kernel

    def patched(bir_json, tmpdir, neff_name="file.neff"):
        return orig(_split_waits(bir_json), tmpdir, neff_name=neff_name)

    bu.compile_bir_kernel = patched
    b2j.compile_bir_kernel = patched


def _build_nc():
    import concourse.bass as bass
    import concourse.mybir as mybir
    from concourse.tile import TileContext

    bf = mybir.dt.bfloat16
    f32 = mybir.dt.float32
    MUL = mybir.AluOpType.mult
    SUB = mybir.AluOpType.subtract
    ADD = mybir.AluOpType.add
    Act = mybir.ActivationFunctionType

    nc = bass.Bass()
    xT_d = nc.dram_tensor("xT", (E, S), bf, kind="ExternalInput")
    wqk_d = nc.dram_tensor("wqk", (E, 4 * 128), bf, kind="ExternalInput")
    wv_d = nc.dram_tensor("wv", (E, HC * 64), bf, kind="ExternalInput")
    wg_d = nc.dram_tensor("wg", (E, HC * 64), bf, kind="ExternalInput")
    wo_d = nc.dram_tensor("wo", (128, 2, E), bf, kind="ExternalInput")
    cs_d = nc.dram_tensor("cs", (128, S), bf, kind="ExternalInput")
    sn_d = nc.dram_tensor("sn", (128, S), bf, kind="ExternalInput")
    out_d = nc.dram_tensor("out", (S, E), f32, kind="ExternalOutput")

    scale = float(D) ** -0.5

    with TileContext(nc) as tc:
        with (
            tc.tile_pool(name="const", bufs=1) as cpool,
            tc.tile_pool(name="big", bufs=1) as bpool,
            tc.tile_pool(name="work", bufs=3) as wpool,
            tc.tile_pool(name="rope", bufs=1) as rpool,
            tc.tile_pool(name="expool", bufs=14) as expool,
            tc.tile_pool(name="outp", bufs=2) as opool,
            tc.tile_pool(name="scp", bufs=2, space="PSUM") as scpool,
            tc.tile_pool(name="pvp", bufs=1, space="PSUM") as pvpool,
        ):
            # ---- persistent tiles ----
            wqk = cpool.tile([128, KT, 512], bf)
            xT = bpool.tile([128, KT, S], bf)
            cs = cpool.tile([128, S], bf)
            sn = cpool.tile([128, S], bf)
            wv = cpool.tile([128, KT, 256], bf)
            wg = cpool.tile([128, KT, 256], bf)
            wo = cpool.tile([128, 2, E], bf)
            scr = cpool.tile([128, 512], bf)   # never written: PE warm-up fuel
            qR = [bpool.tile([128, S], bf, tag=f"qR{g}", name=f"qR{g}") for g in range(2)]
            kR = [bpool.tile([128, S], bf, tag=f"kR{g}", name=f"kR{g}") for g in range(2)]
            gP = [bpool.tile([128, S], bf, tag=f"gP{g}", name=f"gP{g}") for g in range(2)]
            ag = [bpool.tile([128, S], bf, tag=f"ag{g}", name=f"ag{g}") for g in range(2)]
            vOnes = [bpool.tile([128, HC * 65], bf, tag=f"vo{s}", name=f"vo{s}")
                     for s in range(ST)]

            # ---- PE warm-up: keep the HAM clock gate open until real MMs ----
            wps = scpool.tile([128, 512], f32, tag="sc", name="warm_mm")
            nc.gpsimd.memset(scr, 0.0)
            for i in range(32):
                nc.tensor.matmul(wps, lhsT=scr[:, 0:128], rhs=scr,
                                 start=(i == 0), stop=(i == 31))

            # ---- input DMAs, ordered so the q projection can start ASAP ----
            nc.sync.dma_start(wqk[:, :, 0:512],
                              wqk_d[:, :].rearrange("(k p) m -> p k m", p=128))
            nc.sync.dma_start(xT[:, :, 0:1024],
                              xT_d[:, 0:1024].rearrange("(k p) s -> p k s", p=128))
            nc.sync.dma_start(cs[:, 0:1024], cs_d[:, 0:1024])
            nc.sync.dma_start(sn[:, 0:1024], sn_d[:, 0:1024])
            nc.sync.dma_start(xT[:, :, 1024:2048],
                              xT_d[:, 1024:2048].rearrange("(k p) s -> p k s", p=128))
            nc.sync.dma_start(cs[:, 1024:2048], cs_d[:, 1024:2048])
            nc.sync.dma_start(sn[:, 1024:2048], sn_d[:, 1024:2048])
            nc.sync.dma_start(wv, wv_d[:, :].rearrange("(k p) m -> p k m", p=128))
            nc.sync.dma_start(wg, wg_d[:, :].rearrange("(k p) m -> p k m", p=128))
            nc.sync.dma_start(wo, wo_d[:, :, :])

            # warm the ACT exp/tanh table set before it is on the critical path
            warm = cpool.tile([1, 8], f32)
            nc.vector.memset(warm, 0.0)
            nc.scalar.activation(warm, warm, Act.Exp)

            # one-hot rows for the matmul-based partition broadcast of the
            # softmax denominators: hot[32h, 128h + r] = 1
            hot = cpool.tile([128, HC * 128], bf)
            nc.vector.memset(hot, 0.0)
            for h in range(HC):
                nc.vector.memset(hot[32 * h:32 * h + 1, 128 * h:128 * (h + 1)], 1.0)

            # ---- emit helpers ----

            def emit_qk_slice(which, ssl):
                """Project+RoPE+assemble q or k for one 512-col seq slice.
                The two m-tiles' accumulation chains are interleaved so each
                matmul's LDWEIGHTS can prefetch into the background weight
                buffer while the other chain's matmul streams."""
                mbase = 0 if which == "q" else 2
                L = ssl.stop - ssl.start
                pss = [scpool.tile([128, L], f32, tag="sc", name=f"ps_{which}{mi}")
                       for mi in range(2)]
                for k in range(KT):
                    for o in range(0, L, 512):
                        for mi in range(2):
                            nc.tensor.matmul(
                                pss[mi][:, o:o + 512],
                                lhsT=wqk[:, k,
                                         (mbase + mi) * 128:(mbase + mi + 1) * 128],
                                rhs=xT[:, k, ssl.start + o:ssl.start + o + 512],
                                start=(k == 0), stop=(k == KT - 1),
                            )
                raw = []
                for mi in range(2):
                    r = rpool.tile([128, L], bf, tag=f"raw{which}{mi}",
                                   name=f"raw{which}{mi}")
                    nc.vector.tensor_copy(r, pss[mi])
                    raw.append(r)
                ev, od = raw
                top = rpool.tile([128, L], bf, tag=f"top{which}", name=f"top{which}")
                bot = rpool.tile([128, L], bf, tag=f"bot{which}", name=f"bot{which}")
                t1 = wpool.tile([128, L], bf, tag="rt1", name="rt1", bufs=2)
                t2 = wpool.tile([128, L], bf, tag="rt2", name="rt2", bufs=2)
                nc.vector.tensor_tensor(t1, ev, cs[:, ssl], MUL)
                nc.vector.tensor_tensor(t2, od, sn[:, ssl], MUL)
                nc.vector.tensor_tensor(top, t1, t2, SUB)
                t3 = wpool.tile([128, L], bf, tag="rt1", name="rt3", bufs=2)
                t4 = wpool.tile([128, L], bf, tag="rt2", name="rt4", bufs=2)
                nc.vector.tensor_tensor(t3, ev, sn[:, ssl], MUL)
                nc.vector.tensor_tensor(t4, od, cs[:, ssl], MUL)
                nc.vector.tensor_tensor(bot, t3, t4, ADD)
                # assemble into qR/kR pair layout: rows 64*h2+{0:32 top, 32:64 bot}
                dsts = qR if which == "q" else kR
                for g in range(2):
                    for h2 in range(2):
                        h = 2 * g + h2
                        nc.sync.dma_start(dsts[g][64 * h2:64 * h2 + 32, ssl],
                                          top[32 * h:32 * h + 32, :])
                        nc.sync.dma_start(dsts[g][64 * h2 + 32:64 * h2 + 64, ssl],
                                          bot[32 * h:32 * h + 32, :])

            def emit_v(s0):
                """v projection for seq tiles (s0, s0+1) into [v_h | 1]
                stationary tiles; the two k-chains interleave for LDWEIGHTS
                background prefetch."""
                pair = (s0, s0 + 1)
                pss = []
                for s in pair:
                    nc.gpsimd.memset(vOnes[s], 1.0)
                    pss.append(scpool.tile([128, 512], f32, tag="sc", name="ps_v"))
                for k in range(KT):
                    for i, s in enumerate(pair):
                        nc.tensor.matmul(
                            pss[i][:, 0:256],
                            lhsT=xT[:, k, s * 128:(s + 1) * 128],
                            rhs=wv[:, k, :],
                            start=(k == 0), stop=(k == KT - 1),
                        )
                for i, s in enumerate(pair):
                    nc.vector.tensor_copy(
                        vOnes[s].rearrange("p (h w) -> p h w", w=65)[:, :, 0:64],
                        pss[i][:, 0:256].rearrange("p (h w) -> p h w", w=64),
                    )

            def emit_gate(g, n2):
                """gate projection (sigmoid via tanh) for one 1024-col slice.
                k-outer/half-inner so the stationary is reused by 2 matmuls."""
                sl = slice(n2 * 1024, (n2 + 1) * 1024)
                ps = scpool.tile([128, 1024], f32, tag="sc", name="ps_g")
                for k in range(KT):
                    for half in range(2):
                        o = 512 * half
                        nc.tensor.matmul(
                            ps[:, o:o + 512],
                            lhsT=wg[:, k, g * 128:(g + 1) * 128],
                            rhs=xT[:, k, n2 * 1024 + o:n2 * 1024 + o + 512],
                            start=(k == 0), stop=(k == KT - 1),
                        )
                th = wpool.tile([128, 1024], bf, tag="th", name="th", bufs=1)
                nc.scalar.activation(th, ps, Act.Tanh, scale=0.5)
                # sigmoid(x) = 0.5*tanh(x/2) + 0.5
                nc.vector.tensor_scalar(gP[g][:, sl], th, 0.5, 0.5, MUL, ADD)

            def emit_score_exp(c, sk):
                """scores+exp for one (chunk, key-tile); returns the ex pair."""
                csl = slice(c * SQ, (c + 1) * SQ)
                exs = []
                for g in range(2):
                    sct = scpool.tile([128, 1024], f32, tag="sc", name="sct")
                    for h2 in range(2):
                        nc.tensor.matmul(
                            sct[:, h2 * 512:(h2 + 1) * 512],
                            lhsT=kR[g][64 * h2:64 * (h2 + 1), sk * 128:(sk + 1) * 128],
                            rhs=qR[g][64 * h2:64 * (h2 + 1), csl],
                            start=True, stop=True,
                        )
                    ex = expool.tile([128, 1024], bf, tag="ex", name="ex")
                    nc.scalar.activation(ex, sct, Act.Exp, scale=scale)
                    exs.append(ex)
                return exs

            def emit_pv(sk, pv, exs):
                for g in range(2):
                    for h2 in range(2):
                        h = 2 * g + h2
                        nc.tensor.matmul(
                            pv[h][0:65, :],
                            lhsT=vOnes[sk][:, h * 65:(h + 1) * 65],
                            rhs=exs[g][:, h2 * 512:(h2 + 1) * 512],
                            start=(sk == 0), stop=(sk == ST - 1),
                        )

            def emit_att(c, sk, pv):
                emit_pv(sk, pv, emit_score_exp(c, sk))

            def emit_norm(c, pv, tail=False):
                """normalize + gate for one chunk -> ag pair tiles.
                Constraints honoured: engine ops need base partitions in
                {0,32,64,96}; both-SBUF inputs must share a base partition; at
                most one PSUM input.  The uu copies evacuate + release the pv
                banks first so the next chunk's PV accumulation restarts fast;
                recB reuses the pv-tag banks (the expool runway rides the gap).
                The reciprocal broadcast runs as two bf16 matmuls (hi + lo
                residual), the manual version of the fp32 LOW_HIGH split."""
                csl = slice(c * SQ, (c + 1) * SQ)
                dstack = wpool.tile([128, SQ], f32, tag="dstack", name="dstack", bufs=2)
                nc.vector.memset(dstack, 1.0)
                for h in range(HC):
                    # at the kernel tail ScalarE is idle: offload the small
                    # denominator-row moves there so DVE starts uu sooner
                    if tail and h % 2 == 1:
                        nc.scalar.copy(dstack[32 * h:32 * h + 1, :], pv[h][64:65, :])
                    else:
                        nc.vector.tensor_copy(dstack[32 * h:32 * h + 1, :],
                                              pv[h][64:65, :])
                uus = []
                for h in range(HC):
                    o = 64 * (h % 2)
                    uu = wpool.tile([128, SQ], f32, tag=f"uu{h}", name=f"uu{h}", bufs=1)
                    if tail and h >= 2:
                        nc.scalar.copy(uu[o:o + 64, :], pv[h][0:64, :])
                    else:
                        nc.vector.tensor_copy(uu[o:o + 64, :], pv[h][0:64, :])
                    uus.append(uu)
                rec128 = wpool.tile([128, SQ], f32, tag="rec128", name="rec128", bufs=2)
                nc.vector.reciprocal(out=rec128, in_=dstack)
                rhi = wpool.tile([128, SQ], bf, tag="rhi", name="rhi", bufs=2)
                rlo = wpool.tile([128, SQ], bf, tag="rlo", name="rlo", bufs=2)
                nc.vector.tensor_copy(rhi, rec128)
                nc.vector.tensor_tensor(rlo, rec128, rhi, SUB)
                for h in range(HC):
                    g, h2 = divmod(h, 2)
                    o = 64 * h2
                    recB = pvpool.tile([128, SQ], f32, tag=f"pv{h}", name="recB")
                    nc.tensor.matmul(recB, lhsT=hot[:, 128 * h:128 * (h + 1)],
                                     rhs=rhi, start=True, stop=False)
                    nc.tensor.matmul(recB, lhsT=hot[:, 128 * h:128 * (h + 1)],
                                     rhs=rlo, start=False, stop=True)
                    tmpu = wpool.tile([128, SQ], f32, tag="tmpu", name="tmpu", bufs=2)
                    nc.vector.tensor_tensor(tmpu[o:o + 64, :], uus[h][o:o + 64, :],
                                            recB[0:64, :], MUL)
                    nc.vector.tensor_tensor(
                        ag[g][o:o + 64, csl],
                        tmpu[o:o + 64, :],
                        gP[g][o:o + 64, csl],
                        MUL,
                    )

            def emit_out(c, tail=False):
                """output projection for one chunk's 4 row-tiles."""
                for st in range(SQ // 128):
                    s = (SQ // 128) * c + st
                    obs = [pvpool.tile([128, SQ], f32, tag=f"pv{n}", name="ps_o")
                           for n in range(2)]
                    for g in range(2):
                        for n in range(2):
                            nc.tensor.matmul(
                                obs[n],
                                lhsT=ag[g][:, s * 128:(s + 1) * 128],
                                rhs=wo[:, g, n * 512:(n + 1) * 512],
                                start=(g == 0), stop=(g == 1),
                            )
                    ob = opool.tile([128, 1024], f32, tag="ob", name="ob")
                    if tail:
                        nc.vector.tensor_copy(ob[:, 0:512], obs[0])
                        nc.scalar.copy(ob[:, 512:1024], obs[1])
                    else:
                        nc.vector.tensor_copy(ob[:, 0:512], obs[0])
                        nc.vector.tensor_copy(ob[:, 512:1024], obs[1])
                    nc.sync.dma_start(out_d[s * 128:(s + 1) * 128, :], ob)

            # ---- emission schedule ----
            # serial projection phase (PE-dense, streams best), then pure
            # attention chunks.  The one structural overlap that empirically
            # pays: each chunk's first 5 score/exp tiles are emitted BEFORE the
            # previous chunk's normalize/out-proj, so the static PE stream
            # keeps feeding ScalarE across the boundary while the pv banks
            # hand off pv(c-1) -> recB(c-1) -> ps_o(c-1) -> pv(c).
            emit_qk_slice("q", slice(0, 1024))
            emit_qk_slice("q", slice(1024, 2048))
            emit_qk_slice("k", slice(0, 1024))
            emit_qk_slice("k", slice(1024, 2048))
            for s0 in range(0, ST, 2):
                emit_v(s0)
            for g, n2 in ((0, 0), (1, 0), (0, 1), (1, 1)):
                emit_gate(g, n2)

            pv_c = [pvpool.tile([128, SQ], f32, tag=f"pv{h}", name=f"pv{h}")
                    for h in range(HC)]
            for sk in range(ST):
                emit_att(0, sk, pv_c)
            for c in range(1, NCH):
                exs = [emit_score_exp(c, sk) for sk in range(5)]
                emit_norm(c - 1, pv_c)
                emit_out(c - 1)
                pv_n = [pvpool.tile([128, SQ], f32, tag=f"pv{h}", name=f"pv{h}")
                        for h in range(HC)]
                for i in range(5):
                    emit_pv(i, pv_n, exs[i])
                for sk in range(5, ST):
                    emit_att(c, sk, pv_n)
                pv_c = pv_n
            emit_norm(NCH - 1, pv_c, tail=True)
            emit_out(NCH - 1, tail=True)

    return nc


def _host_inputs(x, w_qkv, w_gate, w_out):
    """Build the 8 per-core input maps (all device tensors bf16)."""
    bf = ml_dtypes.bfloat16
    x = np.asarray(x, dtype=np.float32)
    w_qkv = np.asarray(w_qkv, dtype=np.float32)
    w_gate = np.asarray(w_gate, dtype=np.float32)
    w_out = np.asarray(w_out, dtype=np.float32)

    inv = 1.0 / (ROPE_THETA ** (np.arange(0, D, 2, dtype=np.float64) / D))   # [32]
    ang = np.arange(S, dtype=np.float64)[None, :] * inv[:, None]             # [32, S]
    cs = np.tile(np.cos(ang), (4, 1)).astype(bf)                             # [128, S]
    sn = np.tile(np.sin(ang), (4, 1)).astype(bf)

    wq = w_qkv[:, 0:E]
    wk = w_qkv[:, E:2 * E]
    wvv = w_qkv[:, 2 * E:3 * E]

    in_maps = []
    for c in range(NCORES):
        b = c // 4
        hs = HC * (c % 4)
        cols_ev = np.concatenate([(hs + h) * 64 + np.arange(0, 64, 2) for h in range(HC)])
        cols_od = cols_ev + 1
        wqk_p = np.concatenate(
            [wq[:, cols_ev], wq[:, cols_od], wk[:, cols_ev], wk[:, cols_od]], axis=1)
        vcols = np.concatenate([(hs + h) * 64 + np.arange(64) for h in range(HC)])
        wo_p = w_out[vcols, :].reshape(2, 128, E).transpose(1, 0, 2)
        in_maps.append({
            "xT": np.ascontiguousarray(x[b].T).astype(bf),
            "wqk": np.ascontiguousarray(wqk_p).astype(bf),
            "wv": np.ascontiguousarray(wvv[:, vcols]).astype(bf),
            "wg": np.ascontiguousarray(w_gate[:, vcols]).astype(bf),
            "wo": np.ascontiguousarray(wo_p).astype(bf),
            "cs": cs,
            "sn": sn,
        })
    return in_maps


def kernel(x, w_qkv, w_gate, w_out, b_out, n_heads):
    global LAST_RESULTS
    assert int(n_heads) == H
    x = np.asarray(x)
    assert x.shape == (B, S, E)

    from concourse.bass_utils import run_bass_kernel_spmd

    _install_birfix()
    if "nc" not in _CACHE:
        _CACHE["nc"] = _build_nc()
    nc = _CACHE["nc"]

    in_maps = _host_inputs(x, w_qkv, w_gate, w_out)
    import os
    trace = bool(int(os.environ.get("KERNEL_TRACE", "0")))
    tmpdir = os.environ.get("KERNEL_TRACE_DIR") if trace else None
    res = run_bass_kernel_spmd(nc, in_maps, list(range(NCORES)), trace=trace,
                               tmpdir=tmpdir)
    LAST_RESULTS = res

    out = np.zeros((B, S, E), dtype=np.float32)
    for c in range(NCORES):
        out[c // 4] += res.results[c]["out"]
    out += np.asarray(b_out, dtype=np.float32)[None, None, :]
    return out
